# revision 40
# baseline (speedup 1.0000x reference)
"""Multi-head attention (B=2, L=2048, D=1024, H=16) on 8 TRN2 NeuronCores.

Sharding: core c handles batch b = c//4 and head group g = c%4 (4 heads,
256 features). No inter-core communication; host sums the 4 per-head-group
output partials per batch and adds bo.

Per-core schedule (engine-balanced software pipeline):
  - warmup matmuls on a memset tile cover the ~9us framework/DMA startup and
    hold the PE HAM clock-gate at 2.4GHz; a tiny exp() preloads the ACT table
    and a dummy partition_broadcast preloads the GpSimd ucode library
  - input DMAs split across the Sync and GpSimd DGEs: xk on Sync (k proj is
    the scores gate), xq on GpSimd, xv halves on both
  - k projection (DMA-paced, d-outer), then q projection for query-block 0
    only; remaining q chunks and the v projection interleave with block-0
    scores so ScalarE starts exp'ing ~32us in
  - attention runs as 8 half-blocks (512 queries x head-pair). Per key tile m:
    the two heads' scores matmuls (K=64) issue back-to-back at PE row groups
    (0,0)/(64,0) so they stream concurrently in the array; one [128,1024] exp
    on ScalarE; two accumulating attn@V chain matmuls (ones-column emits the
    softmax denominators). The sc PSUM ring paces the PE to ScalarE's rate.
  - normalization: DVE reciprocal + GpSimd partition_broadcast + DVE multiply
    (no PE broadcast matmuls); output projection per query block with PSUM
    evacuation alternating between ScalarE and VectorE and output DMAs
    alternating between both DGE queues.
"""

import math
import sys

sys.path.insert(0, "/opt/trn_rl_repo")

import ml_dtypes
import numpy as np

import concourse.bass as bass
import concourse.mybir as mybir
import concourse.tile as tile
from concourse.tile_rust import add_dep_helper
from concourse import bacc
from concourse.bass_utils import run_bass_kernel_spmd

B, L, D, H, DH = 2, 2048, 1024, 16, 64
NCORES = 8
HPC = 4                  # heads per core
FPC = HPC * DH           # 256 features per core
ND = D // 128            # 8 contraction tiles
NFT = FPC // 128         # 2 feature tiles for q/k/ctx
NM = L // 128            # 16 key tiles
VW = DH + 1              # 65 = head block width in v (64 feats + ones col)
VROW = HPC * VW          # 260
NQB = 4                  # 512-query blocks
SCALE = 1.0 / math.sqrt(DH)
CDT = mybir.dt.bfloat16
NP_CDT = ml_dtypes.bfloat16
F32 = mybir.dt.float32
EXP = mybir.ActivationFunctionType.Exp
OUT_NAME = "outT"
# half-blocks: (query block, head pair)
HB = [(qb, hp) for qb in range(NQB) for hp in range(2)]

_CACHE = {}


def build_nc():
    nc = bacc.Bacc(
        "TRN2",
        target_bir_lowering=False,
        debug=False,
        enable_asserts=False,
        num_devices=NCORES,
    )
    # weights and xq are host-arranged partition-major so each loads with
    # one large-descriptor DMA (512B descriptors measured ~20us for 0.5MB)
    xqT_d = nc.dram_tensor("xqT", [128, NQB, ND, 512], CDT, kind="ExternalInput")
    xkT_d = nc.dram_tensor("xkT", [D, L], CDT, kind="ExternalInput")
    xvT_d = nc.dram_tensor("xvT", [D, L], CDT, kind="ExternalInput")
    wq_d = nc.dram_tensor("wqT", [128, ND, FPC], CDT, kind="ExternalInput")
    wk_d = nc.dram_tensor("wkT", [128, ND, FPC], CDT, kind="ExternalInput")
    wv_d = nc.dram_tensor("wvT", [128, ND, VROW], CDT, kind="ExternalInput")
    wo_d = nc.dram_tensor("woT", [128, NFT, D], CDT, kind="ExternalInput")
    bq_d = nc.dram_tensor("bq2", [128, NFT], F32, kind="ExternalInput")
    bk_d = nc.dram_tensor("bk2", [128, NFT], F32, kind="ExternalInput")
    bvb_d = nc.dram_tensor("bvb", [128, VROW], F32, kind="ExternalInput")
    out_d = nc.dram_tensor(OUT_NAME, [D, L], CDT, kind="ExternalOutput")

    with tile.TileContext(nc) as tc:
        with tc.tile_pool(name="persist", bufs=1) as pp:
            qT = pp.tile([128, NFT, L], CDT)
            kT = pp.tile([128, NFT, L], CDT)
            vsb = pp.tile([128, NM, VROW], CDT)
            ctxT = pp.tile([128, NFT, L], CDT)
            wo_sb = pp.tile([128, NFT, D], CDT)
            bq_sb = pp.tile([128, NFT], F32)
            bk_sb = pp.tile([128, NFT], F32)
            bvb_sb = pp.tile([128, VROW], F32)
            warm = pp.tile([128, 512], CDT)
            actw = pp.tile([1, 16], F32)
            bcw_in = pp.tile([1, 16], F32)
            bcw = pp.tile([64, 16], F32)

            nc.vector.memset(warm[:], 0.25)
            nc.vector.memset(bcw_in[:], 1.0)
            # preload the exp ACT table set during the DMA-wait window
            nc.scalar.activation(actw[:], warm[0:1, 0:16], EXP, scale=SCALE)

            with tc.tile_pool(name="stageV", bufs=1) as sv:
                wv_sb = sv.tile([128, ND, VROW], CDT)
                xv_sb = sv.tile([128, ND, L], CDT)
                xv_r = xvT_d.rearrange("(n p) l -> p n l", p=128)

                with tc.tile_pool(name="stageQK", bufs=1) as sq:
                    wk_sb = sq.tile([128, ND, FPC], CDT)
                    xk_sb = sq.tile([128, ND, L], CDT)
                    wq_sb = sq.tile([128, ND, FPC], CDT)
                    xq_sb = sq.tile([128, NQB, ND, 512], CDT)
                    xk_r = xkT_d.rearrange("(n p) l -> p n l", p=128)
                    # Both DGE queues share the 16 DMA engines, so arrival
                    # order == issue order by priority: xk (gates scores),
                    # then wq + xq's first query block (block-0 scores), then
                    # wv+xv, then the rest of xq, then wo. Even d-slices on
                    # Sync, odd on GpSimd.
                    nc.gpsimd.dma_start(bq_sb[:], bq_d[:])
                    nc.gpsimd.dma_start(bk_sb[:], bk_d[:])
                    nc.gpsimd.dma_start(bvb_sb[:], bvb_d[:])
                    nc.sync.dma_start(wk_sb[:], wk_d[:])
                    nc.gpsimd.dma_start(wq_sb[:], wq_d[:])
                    for d in range(ND):
                        eng = nc.sync if d % 2 == 0 else nc.gpsimd
                        eng.dma_start(xk_sb[:, d, :], xk_r[:, d, :])
                    # q inputs for query block 0 (gates the first scores)
                    nc.sync.dma_start(xq_sb[:, 0, 0:4, :], xqT_d[:, 0, 0:4, :])
                    nc.gpsimd.dma_start(xq_sb[:, 0, 4:8, :], xqT_d[:, 0, 4:8, :])
                    nc.sync.dma_start(wv_sb[:], wv_d[:])
                    for d in range(ND):
                        eng = nc.sync if d % 2 == 0 else nc.gpsimd
                        eng.dma_start(xv_sb[:, d, :], xv_r[:, d, :])
                    for ch in range(1, NQB):
                        eng = nc.sync if ch % 2 == 0 else nc.gpsimd
                        eng.dma_start(xq_sb[:, ch, :, :], xqT_d[:, ch, :, :])
                    nc.gpsimd.dma_start(wo_sb[:], wo_d[:])
                    # preload the GpSimd ucode library (LOAD_LIB costs ~7us
                    # of GpSimd queue time; it must precede the first
                    # normalize but must not delay any input descriptors)
                    nc.gpsimd.partition_broadcast(bcw[:], bcw_in[:], channels=64)

                    with tc.tile_pool(name="psW", bufs=1, space="PSUM") as psW:
                        wps = psW.tile([128, 512], F32)
                        # HAM warmup + cover DGE spin-up before inputs land
                        for i in range(22):
                            nc.tensor.matmul(
                                wps[:], warm[:, 0:128], warm[:],
                                start=True, stop=True, skip_group_check=True,
                            )
                        with tc.tile_pool(name="psA", bufs=5, space="PSUM") as psA:
                            # NOTE: the first scores must strictly follow ALL
                            # kT/qT block-0 writes. Any schedule overlapping
                            # scores with the ft=1 projection writes (split
                            # emission, filler units, or ft0-first reordering)
                            # intermittently corrupts qb0 on hardware -- the
                            # slice-level dependency tracking misses the
                            # partition-offset reads. Keep ft-major order.
                            for ft in range(NFT):
                                pss = [
                                    psA.tile([128, 512], F32, tag="pjk",
                                             name=f"pk_{ft}_{ch}")
                                    for ch in range(4)
                                ]
                                for d in range(ND):
                                    for ch in range(4):
                                        nc.tensor.matmul(
                                            pss[ch][:],
                                            wk_sb[:, d, ft * 128:(ft + 1) * 128],
                                            xk_sb[:, d, ch * 512:(ch + 1) * 512],
                                            start=(d == 0),
                                            stop=(d == ND - 1),
                                        )
                                    if ft == 0:
                                        # dep-free filler between DMA-paced
                                        # d-steps keeps the HAM clock-gate at
                                        # 2.4GHz through the k projection
                                        for _ in range(5):
                                            nc.tensor.matmul(
                                                wps[:], warm[:, 0:128], warm[:],
                                                start=True, stop=True,
                                                skip_group_check=True,
                                            )
                                for ch in range(4):
                                    nc.vector.tensor_scalar_add(
                                        kT[:, ft, ch * 512:(ch + 1) * 512],
                                        pss[ch][:],
                                        bk_sb[:, ft:ft + 1],
                                    )
                            pss = [
                                psA.tile([128, 512], F32, tag="pjk",
                                         name=f"pq_{ft}_0")
                                for ft in range(NFT)
                            ]
                            for d in range(ND):
                                for ft in range(NFT):
                                    nc.tensor.matmul(
                                        pss[ft][:],
                                        wq_sb[:, d, ft * 128:(ft + 1) * 128],
                                        xq_sb[:, 0, d, :],
                                        start=(d == 0),
                                        stop=(d == ND - 1),
                                    )
                            for ft in range(NFT):
                                nc.vector.tensor_scalar_add(
                                    qT[:, ft, 0:512],
                                    pss[ft][:],
                                    bq_sb[:, ft:ft + 1],
                                )

                    # ---- attention pipeline ----
                    with (
                        tc.tile_pool(name="probs", bufs=16) as pb,
                        tc.tile_pool(name="smalls", bufs=2) as sm,
                        tc.tile_pool(name="psU", bufs=2, space="PSUM") as psU,
                    ):
                        probs = {}
                        chains = {}
                        anchor = [None]

                        def pin(inst):
                            # order-only edges are PE-queue ordering hints;
                            # only legal between same-engine instructions
                            if (
                                inst is not None
                                and anchor[0] is not None
                                and inst.ins.engine == mybir.EngineType.PE
                            ):
                                add_dep_helper(
                                    inst.ins, anchor[0].ins, sync=False,
                                    reason="pipeline slot order",
                                )
                            return inst

                        def make_score_pair(psS):
                            def score_pair(si, m):
                                qb, hp = HB[si]
                                sc = psS.tile(
                                    [128, 2, 512], F32, tag="sc",
                                    name=f"sc_{si}_{m}",
                                )
                                for hi in range(2):
                                    po = hi * 64
                                    mmi = nc.tensor.matmul(
                                        sc[:, hi, :],
                                        kT[po:po + 64, hp, m * 128:(m + 1) * 128],
                                        qT[po:po + 64, hp, qb * 512:(qb + 1) * 512],
                                        start=True,
                                        stop=True,
                                    )
                                    if hi == 0:
                                        anchor[0] = mmi
                                pr = pb.tile(
                                    [128, 2, 512], CDT, tag="pr",
                                    name=f"pr_{si}_{m}",
                                )
                                nc.scalar.activation(pr[:], sc[:], EXP, scale=SCALE)
                                probs[(si, m)] = pr
                            return score_pair

                        def start_chains(si, pool):
                            for hi in range(2):
                                chains[(si, hi)] = pool.tile(
                                    [VW, 512], F32, tag="ch", name=f"ch_{si}_{hi}"
                                )

                        def chain_m(si, m):
                            qb, hp = HB[si]
                            pr = probs[(si, m)]
                            for hi in range(2):
                                h = 2 * hp + hi
                                mmi = nc.tensor.matmul(
                                    chains[(si, hi)][:],
                                    vsb[:, m, h * VW:(h + 1) * VW],
                                    pr[:, hi, :],
                                    start=(m == 0),
                                    stop=(m == NM - 1),
                                )
                                if hi == 0:
                                    anchor[0] = mmi
                            del probs[(si, m)]

                        def normalize(si):
                            qb, hp = HB[si]
                            # one den|recip|broadcast pass for both heads: the
                            # serial chain is exposed at block boundaries
                            den = sm.tile([1, 2, 512], F32, tag="den", bufs=1,
                                          name=f"den_{si}")
                            ch0 = chains.pop((si, 0))
                            ch1 = chains.pop((si, 1))
                            nc.vector.tensor_copy(den[:, 0, :], ch0[64:65, :])
                            nc.vector.tensor_copy(den[:, 1, :], ch1[64:65, :])
                            rec = sm.tile([1, 2, 512], F32, tag="rec",
                                          name=f"rec_{si}")
                            nc.vector.reciprocal_approx_fast(rec[:], den[:])
                            rbb = sm.tile([64, 2, 512], F32, tag="rbb",
                                          name=f"rbb_{si}")
                            nc.gpsimd.partition_broadcast(rbb[:], rec[:],
                                                          channels=64)
                            for hi, ch in ((0, ch0), (1, ch1)):
                                po = hi * 64
                                nc.vector.tensor_mul(
                                    ctxT[po:po + 64, hp, qb * 512:(qb + 1) * 512],
                                    ch[0:64, :],
                                    rbb[:, hi, :],
                                )

                        pending = []

                        def outproj_unit(qb, ft8):
                            ops = psU.tile(
                                [128, 512], F32, tag="u", name=f"op_{qb}_{ft8}"
                            )
                            for d2 in range(NFT):
                                pin(nc.tensor.matmul(
                                    ops[:],
                                    wo_sb[:, d2, ft8 * 128:(ft8 + 1) * 128],
                                    ctxT[:, d2, qb * 512:(qb + 1) * 512],
                                    start=(d2 == 0),
                                    stop=(d2 == NFT - 1),
                                ))
                            st = sm.tile(
                                [128, 512], CDT, tag="ost", bufs=6,
                                name=f"st_{qb}_{ft8}",
                            )
                            # evacuate on VectorE only: ScalarE is strict
                            # FIFO, so a copy waiting on the out-DMA ring
                            # would block every exp queued behind it
                            nc.vector.tensor_copy(st[:], ops[:])
                            dma_eng = nc.gpsimd if ft8 % 2 == 0 else nc.sync
                            dma_eng.dma_start(
                                out_d[
                                    ft8 * 128:(ft8 + 1) * 128,
                                    qb * 512:(qb + 1) * 512,
                                ],
                                st[:],
                            )

                        # deferred projection chains emitted as PE filler in
                        # the ScalarE-rate slack of the pipeline (all psU)
                        def gen_proj(w_sb, b_sb, dstT, rhs_of, units):
                            for ch, ft in units:
                                ps = psU.tile([128, 512], F32, tag="u",
                                              name=f"pj_{id(dstT) % 97}_{ch}_{ft}")
                                for d in range(ND):
                                    yield lambda ps=ps, d=d, ft=ft, ch=ch: \
                                        nc.tensor.matmul(
                                            ps[:],
                                            w_sb[:, d, ft * 128:(ft + 1) * 128],
                                            rhs_of(d, ch),
                                            start=(d == 0),
                                            stop=(d == ND - 1),
                                        )
                                yield lambda ps=ps, ft=ft, ch=ch: \
                                    nc.vector.tensor_scalar_add(
                                        dstT[:, ft, ch * 512:(ch + 1) * 512],
                                        ps[:],
                                        b_sb[:, ft:ft + 1],
                                    )

                        def gen_v():
                            for kt in range(NM):
                                ps = psU.tile([128, 512], F32, tag="u",
                                              name=f"pv_{kt}")
                                for d in range(ND):
                                    yield lambda ps=ps, d=d, kt=kt: \
                                        nc.tensor.matmul(
                                            ps[:, 0:VROW],
                                            xv_sb[:, d, kt * 128:(kt + 1) * 128],
                                            wv_sb[:, d, :],
                                            start=(d == 0),
                                            stop=(d == ND - 1),
                                        )
                                yield lambda ps=ps, kt=kt: \
                                    nc.vector.tensor_add(
                                        vsb[:, kt, :], ps[:, 0:VROW],
                                        bvb_sb[:],
                                    )

                        def emit_filler(gen, n):
                            for _ in range(n):
                                op = next(gen, None)
                                if op is None:
                                    return
                                pin(op())

                        def chaingen(*gens):
                            for g in gens:
                                yield from g

                        k_rhs = lambda d, ch: xk_sb[:, d, ch * 512:(ch + 1) * 512]
                        q_rhs = lambda d, ch: xq_sb[:, ch, d, :]
                        # ft=1 projections (needed by block 1) first, then v
                        # (needs xv, lands ~mid-stretch), then q block 1
                        filler1 = chaingen(
                            gen_v(),
                            gen_proj(wq_sb, bq_sb, qT, q_rhs, [(1, 0), (1, 1)]),
                        )
                        filler2 = gen_proj(
                            wq_sb, bq_sb, qT, q_rhs,
                            [(2, 0), (2, 1), (3, 0), (3, 1)],
                        )

                        with (
                            tc.tile_pool(name="psS", bufs=2, space="PSUM") as psS,
                            tc.tile_pool(name="psC", bufs=2, space="PSUM") as psC,
                        ):
                            score_pair = make_score_pair(psS)
                            # stretch: block-0 scores paced by ScalarE with
                            # deferred projections as PE filler
                            for m in range(NM):
                                score_pair(0, m)
                                emit_filler(filler1, 6)
                            # block 0 chains + block 1 scores
                            start_chains(0, psC)
                            for m in range(NM):
                                chain_m(0, m)
                                score_pair(1, m)
                                emit_filler(filler1, 6)
                            emit_filler(filler1, 1000)
                            normalize(0)

                            for i in range(2, len(HB)):
                                prev = i - 1
                                start_chains(prev, psC)
                                for m in range(NM):
                                    chain_m(prev, m)
                                    score_pair(i, m)
                                    # spread deferred q projection and the
                                    # previous block's output projection into
                                    # the ScalarE-rate slack
                                    if m % 2 == 0:
                                        if i <= 3:
                                            emit_filler(filler2, 4)
                                    elif pending:
                                        pending.pop(0)()
                                if i == 3:
                                    # q blocks 2/3 must precede their scores
                                    emit_filler(filler2, 1000)
                                normalize(prev)
                                qb, hp = HB[prev]
                                if hp == 1:
                                    for ft8 in range(D // 128):
                                        pending.append(
                                            lambda qb=qb, ft8=ft8:
                                            outproj_unit(qb, ft8)
                                        )
                            # final block
                            last = len(HB) - 1
                            start_chains(last, psC)
                            for m in range(NM):
                                chain_m(last, m)
                                if m % 2 == 1 and pending:
                                    pending.pop(0)()
                            normalize(last)
                            qb, hp = HB[last]
                            for ft8 in range(D // 128):
                                pending.append(
                                    lambda qb=qb, ft8=ft8:
                                    outproj_unit(qb, ft8)
                                )
                            while pending:
                                pending.pop(0)()
    nc.compile()
    return nc
